# revision 1
# baseline (speedup 1.0000x reference)
"""Trainium2 Bass kernel for nn_EquivariantProductBasisBlock.

Math: per (n,c) with x = node_feats[n,c,:] in R^9, one-hot node_attrs:
  f[n,c,dt] = sum_k w3[n,k,c] * <U3sym[dt,:,k], mono3(x)>
            + sum_k w2[n,k,c] * <U2sym[dt,:,k], mono2(x)>
            + sum_k w1[n,k,c] * <U1[dt,:,k], x>
  out = concat_dt(f @ Wlin) / sqrt(C) + sc

Host stages the monomial basis (gather/multiply of x rows) in fp16; the
device runs, per 512-column block (4 node-slots x 128 channels, c-fastest):
  G[124, F] = CFa.T @ MA + CFb.T @ MB      (PE, monomial contraction)
  T1        = G * WE32[elem]               (DVE, c-broadcast affine AP)
  T1u       = U1X * WE1[elem]              (DVE, fp16 2x)
  f[4, F]   = R1.T @ T1 + R2.T @ T1u       (PE, k-reduction)
Nodes are dealt to cores round-robin per element class so the
block->element map is identical on all 8 cores (SPMD-uniform); per-element
k-weights enter via a compile-time WE column slice per block.
Host: final equivariant Linear + sc, inverse permutation.
"""
import sys
import numpy as np

sys.path.insert(0, "/opt/trn_rl_repo")

N, C, I, E = 2048, 128, 9, 10
K3, K2, K1 = 23, 8, 3
NCORES = 8
FB = 512                  # free cols per block
SLOTS_PER_BLK = FB // C   # 4 node-slots per block

TRI3 = [(a, b, c) for a in range(I) for b in range(a, I) for c in range(b, I)]
TRI2 = [(a, b) for a in range(I) for b in range(a, I)]
M2IDX = {ab: r for r, ab in enumerate(TRI2)}
NM3, NM2 = len(TRI3), len(TRI2)           # 165, 45
NC3, NC2, NC1 = 4 * K3, 4 * K2, 4 * K1    # 92, 32, 12
NCOL = NC3 + NC2                          # 124
MAR = 128                                 # monomial tile A rows (m3 0..127)
MBR = NM3 - MAR + NM2                     # 82: m3 128..164 | m2 0..44
DT_LIST = [(0, 0), (1, 0), (1, 1), (1, 2)]

_compiled = {}


def _build_consts(inputs):
    """Coefficient / weight matrices derived from the U/W input tensors."""
    U3s = [np.asarray(inputs["U3_0"]), np.asarray(inputs["U3_1"])]
    U2s = [np.asarray(inputs["U2_0"]), np.asarray(inputs["U2_1"])]
    U1s = [np.asarray(inputs["U1_0"]), np.asarray(inputs["U1_1"])]
    W3s = [np.asarray(inputs["W3_0"]), np.asarray(inputs["W3_1"])]
    W2s = [np.asarray(inputs["W2_0"]), np.asarray(inputs["W2_1"])]
    W1s = [np.asarray(inputs["W1_0"]), np.asarray(inputs["W1_1"])]

    # symmetrized U3/U2 -> CF [mono-row, (dt,k) col]
    CF3 = np.zeros((NM3, NCOL), np.float64)
    CF2 = np.zeros((NM2, NCOL), np.float64)
    tri3_idx = {m: r for r, m in enumerate(TRI3)}
    for di, (s, d) in enumerate(DT_LIST):
        u3 = np.zeros((NM3, K3), np.float64)
        u2 = np.zeros((NM2, K2), np.float64)
        U3 = np.asarray(U3s[s], np.float64)
        U2 = np.asarray(U2s[s], np.float64)
        for p in range(I):
            for q in range(I):
                u2[M2IDX[tuple(sorted((p, q)))]] += U2[d, p, q, :]
                for i in range(I):
                    u3[tri3_idx[tuple(sorted((p, q, i)))]] += U3[d, p, q, i, :]
        CF3[:, di * K3:(di + 1) * K3] = u3
        CF2[:, NC3 + di * K2:NC3 + (di + 1) * K2] = u2

    CFall = np.concatenate([CF3, CF2], axis=0)   # [210, 124]
    S1u = np.zeros((I, NC1), np.float32)         # U1 fold: U1X = S1u.T @ xT
    for di, (s, d) in enumerate(DT_LIST):
        S1u[:, di * K1:(di + 1) * K1] = U1s[s][d, :, :]

    R1 = np.zeros((NCOL, 4), np.float16)
    R2 = np.zeros((NC1, 4), np.float16)
    WE32 = np.zeros((NCOL, E, C), np.float32)
    WE1 = np.zeros((NC1, E, C), np.float32)
    for di, (s, d) in enumerate(DT_LIST):
        R1[di * K3:(di + 1) * K3, di] = 1.0
        R1[NC3 + di * K2:NC3 + (di + 1) * K2, di] = 1.0
        R2[di * K1:(di + 1) * K1, di] = 1.0
        WE32[di * K3:(di + 1) * K3] = W3s[s].transpose(1, 0, 2)
        WE32[NC3 + di * K2:NC3 + (di + 1) * K2] = W2s[s].transpose(1, 0, 2)
        WE1[di * K1:(di + 1) * K1] = W1s[s].transpose(1, 0, 2)

    return {
        "CFa": CFall[:MAR].astype(np.float16),
        "CFb": CFall[MAR:].astype(np.float16),
        "R1": R1, "R2": R2,
        "WE32": WE32.reshape(NCOL, E * C).astype(np.float16),
        "WE1": WE1.reshape(NC1, E * C).astype(np.float16),
        "S1u": S1u,
    }


def _build_nc(eb):
    """Bass program; eb = element id per block (same on all cores)."""
    from concourse import bass, bacc, tile, mybir

    f32 = mybir.dt.float32
    f16 = mybir.dt.float16
    NBLK = len(eb)
    FT = NBLK * FB

    nc = bacc.Bacc(None, target_bir_lowering=False, debug=False)
    ma_d = nc.declare_dram_parameter("MA", [MAR, FT], f16, isOutput=False)
    mb_d = nc.declare_dram_parameter("MB", [MBR, FT], f16, isOutput=False)
    ux_d = nc.declare_dram_parameter("U1X", [NC1, FT], f16, isOutput=False)
    cshapes = {
        "CFa": ([MAR, NCOL], f16), "CFb": ([MBR, NCOL], f16),
        "R1": ([NCOL, 4], f16), "R2": ([NC1, 4], f16),
        "WE32": ([NCOL, E * C], f16), "WE1": ([NC1, E * C], f16),
    }
    cd = {k: nc.declare_dram_parameter(k, shp, dt, isOutput=False)
          for k, (shp, dt) in cshapes.items()}
    f_d = nc.declare_dram_parameter("f", [4, FT], f32, isOutput=True)

    with tile.TileContext(nc) as tc:
        with (
            tc.tile_pool(name="const", bufs=1) as cpool,
            tc.tile_pool(name="mono", bufs=2) as mpool,
            tc.tile_pool(name="work", bufs=4) as wpool,
            tc.tile_pool(name="psum", bufs=3, space=bass.MemorySpace.PSUM) as pp,
        ):
            ct = {}
            for k, (shp, dt) in cshapes.items():
                ct[k] = cpool.tile(shp, dt, tag=k, name=k)
                nc.sync.dma_start(out=ct[k][:], in_=cd[k][:])

            XCHUNK = 8  # blocks per monomial DMA chunk
            for b in range(NBLK):
                if b % XCHUNK == 0:
                    w = min(XCHUNK * FB, FT - b * FB)
                    mac = mpool.tile([MAR, XCHUNK * FB], f16, tag="mac")
                    nc.sync.dma_start(out=mac[:, :w],
                                      in_=ma_d[:, b * FB:b * FB + w])
                    mbc = mpool.tile([MBR, XCHUNK * FB], f16, tag="mbc")
                    nc.sync.dma_start(out=mbc[:, :w],
                                      in_=mb_d[:, b * FB:b * FB + w])
                    uxc = mpool.tile([NC1, XCHUNK * FB], f16, tag="uxc")
                    nc.sync.dma_start(out=uxc[:, :w],
                                      in_=ux_d[:, b * FB:b * FB + w])
                o = (b % XCHUNK) * FB
                ma = mac[:, o:o + FB]
                mb = mbc[:, o:o + FB]
                ux = uxc[:, o:o + FB]

                g = pp.tile([NCOL, FB], f32, tag="g")
                nc.tensor.matmul(g[:], ct["CFa"][:], ma, start=True, stop=False)
                nc.tensor.matmul(g[:], ct["CFb"][:], mb, start=False, stop=True)

                e = eb[b]
                we = ct["WE32"][:, e * C:(e + 1) * C]
                web = we.unsqueeze(1).broadcast_to([NCOL, SLOTS_PER_BLK, C])
                t1 = wpool.tile([NCOL, SLOTS_PER_BLK, C], f16, tag="t1")
                nc.vector.tensor_mul(
                    t1[:],
                    g[:].rearrange("p (n c) -> p n c", n=SLOTS_PER_BLK), web)

                we1 = ct["WE1"][:, e * C:(e + 1) * C]
                we1b = we1.unsqueeze(1).broadcast_to([NC1, SLOTS_PER_BLK, C])
                t1u = wpool.tile([NC1, SLOTS_PER_BLK, C], f16, tag="t1u")
                nc.vector.tensor_mul(
                    t1u[:],
                    ux.rearrange("p (n c) -> p n c", n=SLOTS_PER_BLK), we1b)

                f_ps = pp.tile([4, FB], f32, tag="f")
                nc.tensor.matmul(f_ps[:], ct["R1"][:],
                                 t1[:].rearrange("p n c -> p (n c)"),
                                 start=True, stop=False)
                nc.tensor.matmul(f_ps[:], ct["R2"][:],
                                 t1u[:].rearrange("p n c -> p (n c)"),
                                 start=False, stop=True)

                if b % XCHUNK == 0:
                    fstage = wpool.tile([4, XCHUNK * FB], f32, tag="fstage")
                nc.scalar.copy(fstage[:, o:o + FB], f_ps[:])
                if b % XCHUNK == XCHUNK - 1 or b == NBLK - 1:
                    lo = (b // XCHUNK) * XCHUNK
                    w = (b - lo + 1) * FB
                    nc.sync.dma_start(out=f_d[:, lo * FB:lo * FB + w],
                                      in_=fstage[:, :w])

    nc.compile()
    return nc


def kernel(**inputs):
    from concourse.bass_utils import run_bass_kernel_spmd

    x = np.ascontiguousarray(np.asarray(inputs["node_feats"], np.float32))
    sc = np.asarray(inputs["sc"], np.float32)
    y = np.asarray(inputs["node_attrs"], np.float32)
    Wlin0 = np.asarray(inputs["Wlin0"], np.float32)
    Wlin1 = np.asarray(inputs["Wlin1"], np.float32)

    elem = np.argmax(y, axis=1)
    consts = _build_consts(inputs)

    # deal nodes: element e's nodes round-robin over cores
    count = np.bincount(elem, minlength=E)
    spe = [int(np.ceil(cnt / NCORES)) if cnt else 0 for cnt in count]
    blocks_e = [int(np.ceil(s / SLOTS_PER_BLK)) for s in spe]
    eb = []
    base_slot = []
    for e in range(E):
        base_slot.append(len(eb) * SLOTS_PER_BLK)
        eb.extend([e] * blocks_e[e])
    NBLK = len(eb)
    NSLOT = NBLK * SLOTS_PER_BLK
    FT = NBLK * FB

    order = np.argsort(elem, kind="stable")
    core_of = np.empty(N, np.int64)
    slot_of = np.empty(N, np.int64)
    pos = 0
    for e in range(E):
        idx = order[pos:pos + count[e]]
        pos += count[e]
        for j, n_ in enumerate(idx):
            core_of[n_] = j % NCORES
            slot_of[n_] = base_slot[e] + j // NCORES

    key = tuple(eb)
    if key not in _compiled:
        _compiled[key] = _build_nc(eb)
    nc = _compiled[key]

    ia3 = np.array([a for a, b, c in TRI3])
    ib3 = np.array([b for a, b, c in TRI3])
    ic3 = np.array([c for a, b, c in TRI3])
    ia2 = np.array([a for a, b in TRI2])
    ib2 = np.array([b for a, b in TRI2])

    cdicts = {k: v for k, v in consts.items() if k != "S1u"}
    in_maps = []
    for core in range(NCORES):
        xc = np.zeros((NSLOT, C, I), np.float32)
        mask = core_of == core
        xc[slot_of[mask]] = x[mask]
        xT = xc.transpose(2, 0, 1).reshape(I, FT)          # [9, nc]
        m3 = xT[ia3] * xT[ib3] * xT[ic3]                   # [165, nc]
        m2 = xT[ia2] * xT[ib2]                             # [45, nc]
        m = {
            "MA": m3[:MAR].astype(np.float16),
            "MB": np.concatenate([m3[MAR:], m2], 0).astype(np.float16),
            "U1X": (consts["S1u"].T @ xT).astype(np.float16),
        }
        m.update(cdicts)
        in_maps.append(m)

    res = run_bass_kernel_spmd(nc, in_maps, list(range(NCORES)))
    globals()["LAST_RESULT"] = res
    import os
    nrep = int(os.environ.get("KERNEL_TIME_RUNS", "0"))
    if nrep:
        import time
        times = []
        for _ in range(nrep):
            t0 = time.perf_counter()
            run_bass_kernel_spmd(nc, in_maps, list(range(NCORES)))
            times.append(time.perf_counter() - t0)
        globals()["LAST_TIMES"] = times
    fcores = [np.asarray(r["f"]) for r in res.results]

    f_ncd = np.empty((N, C, 4), np.float32)
    for core in range(NCORES):
        fc = fcores[core].reshape(4, NSLOT, C)
        mask = core_of == core
        f_ncd[mask] = fc[:, slot_of[mask], :].transpose(1, 2, 0)

    inv = np.float32(1.0 / np.sqrt(C))
    y0 = np.einsum("nud,uw->nwd", f_ncd[:, :, :1], Wlin0) * inv
    y1 = np.einsum("nud,uw->nwd", f_ncd[:, :, 1:], Wlin1) * inv
    out = np.concatenate([y0.reshape(N, -1), y1.reshape(N, -1)], axis=-1) + sc
    return out.astype(np.float32)



# revision 8
# speedup vs baseline: 14.5469x; 14.5469x over previous
"""Trainium2 Bass kernel for nn_EquivariantProductBasisBlock.

Math: per (n,c) with x = node_feats[n,c,:] in R^9, one-hot node_attrs:
  f[n,c,dt] = sum_k w3[n,k,c] * <U3sym[dt,:,k], mono3(x)>
            + sum_k w2[n,k,c] * <U2sym[dt,:,k], mono2(x)>
            + sum_k w1[n,k,c] * <U1[dt,:,k], x>
  out = concat_dt(f @ Wlin) / sqrt(C) + sc

v2: only the raw features ship to the device (XT [9, FT] fp16, c-fastest
slot-major columns); the monomial basis is built ON DEVICE per chunk:
  xa2/xb2/xa3  <- stride-0-partition DMA replication of XT rows
  m2 = xa2*xb2 (DVE), xb3 <- SBUF DMA of m2 tail segments, m3 = xa3*xb3
then per 512-column block (4 node-slots x 128 channels):
  G[124, F] = CFa.T @ MA + CFb.T @ MB      (PE, monomial contraction)
  U1X[12,F] = S1u.T @ XT                   (PE)
  T1  = G * WE32[elem], T1u = U1X * WE1[elem]   (DVE, c-broadcast AP)
  f[4, F]   = R1.T @ T1 + R2.T @ T1u       (PE, k-reduction)
All weight/coefficient matrices are embedded in the NEFF via
inline_tensor (Const -> HLO constants; zero per-call transfer), so the
per-dispatch traffic is XT up (0.6MB/core) + f down (0.3MB/core).
Nodes are dealt to cores round-robin per element class so the
block->element map is identical on all 8 cores (SPMD-uniform).
Host: final equivariant Linear + sc, inverse permutation.
"""
import hashlib
import os
import sys
import numpy as np

sys.path.insert(0, "/opt/trn_rl_repo")

N, C, I, E = 2048, 128, 9, 10
K3, K2, K1 = 23, 8, 3
NCORES = 8
FB = 512                  # free cols per block
SLOTS_PER_BLK = FB // C   # 4 node-slots per block
XCHUNK = 8                # blocks per chunk
CH = XCHUNK * FB          # 4096 cols per chunk

TRI3 = [(a, b, c) for a in range(I) for b in range(a, I) for c in range(b, I)]
TRI2 = [(a, b) for a in range(I) for b in range(a, I)]
M2IDX = {ab: r for r, ab in enumerate(TRI2)}
NM3, NM2 = len(TRI3), len(TRI2)           # 165, 45
NC3, NC2, NC1 = 4 * K3, 4 * K2, 4 * K1    # 92, 32, 12
NCOL = NC3 + NC2                          # 124
MAR = 128                                 # m3 rows 0..127 -> MA tile
M3B = NM3 - MAR                           # 37 m3 rows in MB tile
MBR = M3B + NM2                           # 82: m3 128..164 | m2 0..44
DT_LIST = [(0, 0), (1, 0), (1, 1), (1, 2)]

# m3 segment start row per leading index a
START3 = {}
_off = 0
for _a in range(I):
    START3[_a] = _off
    _off += (I - _a) * (I - _a + 1) // 2

# m2 segments: (dst row offset, rows, a)
SEG2 = [(M2IDX[(a, a)], I - a, a) for a in range(I)]
# m3 pieces split at the MA/MB boundary: (dst, dst_off, m2_src_off, rows, a)
M3P = []
for _a in range(I):
    _o3 = START3[_a]
    _ln = (I - _a) * (I - _a + 1) // 2
    _src = M2IDX[(_a, _a)]
    while _ln > 0:
        if _o3 < MAR:
            _take = min(_ln, MAR - _o3)
            M3P.append(("A", _o3, _src, _take, _a))
        else:
            _take = _ln
            M3P.append(("B", _o3 - MAR, _src, _take, _a))
        _o3 += _take
        _src += _take
        _ln -= _take

_compiled = {}


def _build_consts(inputs):
    """Coefficient / weight matrices derived from the U/W input tensors."""
    U3s = [np.asarray(inputs["U3_0"]), np.asarray(inputs["U3_1"])]
    U2s = [np.asarray(inputs["U2_0"]), np.asarray(inputs["U2_1"])]
    U1s = [np.asarray(inputs["U1_0"]), np.asarray(inputs["U1_1"])]
    W3s = [np.asarray(inputs["W3_0"]), np.asarray(inputs["W3_1"])]
    W2s = [np.asarray(inputs["W2_0"]), np.asarray(inputs["W2_1"])]
    W1s = [np.asarray(inputs["W1_0"]), np.asarray(inputs["W1_1"])]

    # symmetrized U3/U2 -> CF [mono-row, (dt,k) col]
    CF3 = np.zeros((NM3, NCOL), np.float64)
    CF2 = np.zeros((NM2, NCOL), np.float64)
    tri3_idx = {m: r for r, m in enumerate(TRI3)}
    for di, (s, d) in enumerate(DT_LIST):
        u3 = np.zeros((NM3, K3), np.float64)
        u2 = np.zeros((NM2, K2), np.float64)
        U3 = np.asarray(U3s[s], np.float64)
        U2 = np.asarray(U2s[s], np.float64)
        for p in range(I):
            for q in range(I):
                u2[M2IDX[tuple(sorted((p, q)))]] += U2[d, p, q, :]
                for i in range(I):
                    u3[tri3_idx[tuple(sorted((p, q, i)))]] += U3[d, p, q, i, :]
        CF3[:, di * K3:(di + 1) * K3] = u3
        CF2[:, NC3 + di * K2:NC3 + (di + 1) * K2] = u2

    # device MA = m3[0:128]; MB3 = m3[128:165]; M2 = m2[0:45]
    CFa = CF3[:MAR]
    CFb3 = CF3[MAR:]                                 # [37, 124]
    CF2c = CF2                                       # [45, 124]

    S1u = np.zeros((I, NC1), np.float32)             # U1X = S1u.T @ XT
    for di, (s, d) in enumerate(DT_LIST):
        S1u[:, di * K1:(di + 1) * K1] = U1s[s][d, :, :]

    R1 = np.zeros((NCOL, 4), np.float16)
    R2 = np.zeros((NC1, 4), np.float16)
    WE32 = np.zeros((NCOL, E, C), np.float32)
    WE1 = np.zeros((NC1, E, C), np.float32)
    for di, (s, d) in enumerate(DT_LIST):
        R1[di * K3:(di + 1) * K3, di] = 1.0
        R1[NC3 + di * K2:NC3 + (di + 1) * K2, di] = 1.0
        R2[di * K1:(di + 1) * K1, di] = 1.0
        WE32[di * K3:(di + 1) * K3] = W3s[s].transpose(1, 0, 2)
        WE32[NC3 + di * K2:NC3 + (di + 1) * K2] = W2s[s].transpose(1, 0, 2)
        WE1[di * K1:(di + 1) * K1] = W1s[s].transpose(1, 0, 2)

    return {
        "CFa": CFa.astype(np.float16),
        "CFb3": CFb3.astype(np.float16),
        "CF2c": CF2c.astype(np.float16),
        "R1": R1, "R2": R2,
        "WE32": WE32.reshape(NCOL, E * C).astype(np.float16),
        "WE1": WE1.reshape(NC1, E * C).astype(np.float16),
        "S1u": S1u.astype(np.float16),
    }


def _build_nc(eb, consts):
    """Bass program; eb = element id per block (same on all cores),
    len(eb) divisible by XCHUNK. Consts are baked into the NEFF."""
    from concourse import bass, bacc, tile, mybir

    f32 = mybir.dt.float32
    f16 = mybir.dt.float16
    NBLK = len(eb)
    FT = NBLK * FB
    NCHUNK = NBLK // XCHUNK

    nc = bacc.Bacc(None, target_bir_lowering=False, debug=False)
    xt_d = nc.declare_dram_parameter("XT", [I, FT], f16, isOutput=False)
    f_d = nc.declare_dram_parameter("f", [4, FT], f16, isOutput=True)
    cd = {k: nc.inline_tensor(np.ascontiguousarray(v), name=k)
          for k, v in consts.items()}

    with tile.TileContext(nc) as tc:
        with (
            tc.tile_pool(name="const", bufs=1) as cpool,
            tc.tile_pool(name="mono", bufs=2) as mpool,
            tc.tile_pool(name="work", bufs=2) as wpool,
            tc.tile_pool(name="pg", bufs=2, space=bass.MemorySpace.PSUM) as pg,
            tc.tile_pool(name="pu", bufs=2, space=bass.MemorySpace.PSUM) as pu,
            tc.tile_pool(name="pf", bufs=2, space=bass.MemorySpace.PSUM) as pf,
        ):
            ct = {}
            for k, v in consts.items():
                ct[k] = cpool.tile(list(v.shape), f16, tag=k, name=k)
                nc.sync.dma_start(out=ct[k][:], in_=cd[k][:])

            for ci in range(NCHUNK):
                cs = ci * CH
                xt = mpool.tile([I, CH], f16, tag="xt")
                nc.sync.dma_start(out=xt[:], in_=xt_d[:, cs:cs + CH])

                # pairwise monomials m2 -> mb rows [M3B:]
                xa2 = mpool.tile([NM2, CH], f16, tag="xa2")
                xb2 = mpool.tile([NM2, CH], f16, tag="xb2")
                for (o2, n2, a) in SEG2:
                    nc.sync.dma_start(
                        out=xa2[o2:o2 + n2, :],
                        in_=xt_d[a:a + 1, cs:cs + CH].broadcast_to([n2, CH]))
                    nc.sync.dma_start(
                        out=xb2[o2:o2 + n2, :],
                        in_=xt_d[a:I, cs:cs + CH])
                m2t = mpool.tile([NM2, CH], f16, tag="m2t")
                nc.vector.tensor_mul(m2t[:], xa2[:], xb2[:])

                # cubic monomials m3 = bcast(x_a) * m2-tail
                ma = mpool.tile([MAR, CH], f16, tag="ma")
                mb3 = mpool.tile([M3B, CH], f16, tag="mb3")
                xa3a = mpool.tile([MAR, CH], f16, tag="xa3a")
                xb3a = mpool.tile([MAR, CH], f16, tag="xb3a")
                xa3b = mpool.tile([M3B, CH], f16, tag="xa3b")
                xb3b = mpool.tile([M3B, CH], f16, tag="xb3b")
                for (dst, doff, soff, ln, a) in M3P:
                    xa3 = xa3a if dst == "A" else xa3b
                    xb3 = xb3a if dst == "A" else xb3b
                    nc.sync.dma_start(
                        out=xa3[doff:doff + ln, :],
                        in_=xt_d[a:a + 1, cs:cs + CH].broadcast_to([ln, CH]))
                    nc.sync.dma_start(
                        out=xb3[doff:doff + ln, :],
                        in_=m2t[soff:soff + ln, :])
                nc.vector.tensor_mul(ma[:], xa3a[:], xb3a[:])
                nc.vector.tensor_mul(mb3[:], xa3b[:], xb3b[:])

                fstage = wpool.tile([4, CH], f16, tag="fstage")
                for j in range(XCHUNK):
                    b = ci * XCHUNK + j
                    o = j * FB
                    e = eb[b]

                    g = pg.tile([NCOL, FB], f32, tag="g")
                    nc.tensor.matmul(g[:], ct["CFa"][:], ma[:, o:o + FB],
                                     start=True, stop=False)
                    nc.tensor.matmul(g[:], ct["CFb3"][:], mb3[:, o:o + FB],
                                     start=False, stop=False)
                    nc.tensor.matmul(g[:], ct["CF2c"][:], m2t[:, o:o + FB],
                                     start=False, stop=True)
                    u1 = pu.tile([NC1, FB], f32, tag="u1")
                    nc.tensor.matmul(u1[:], ct["S1u"][:], xt[:, o:o + FB],
                                     start=True, stop=True)

                    we = ct["WE32"][:, e * C:(e + 1) * C]
                    web = we.unsqueeze(1).broadcast_to([NCOL, SLOTS_PER_BLK, C])
                    t1 = wpool.tile([NCOL, SLOTS_PER_BLK, C], f16, tag="t1")
                    nc.vector.tensor_mul(
                        t1[:],
                        g[:].rearrange("p (n c) -> p n c", n=SLOTS_PER_BLK), web)

                    we1 = ct["WE1"][:, e * C:(e + 1) * C]
                    we1b = we1.unsqueeze(1).broadcast_to([NC1, SLOTS_PER_BLK, C])
                    t1u = wpool.tile([NC1, SLOTS_PER_BLK, C], f16, tag="t1u")
                    nc.vector.tensor_mul(
                        t1u[:],
                        u1[:].rearrange("p (n c) -> p n c", n=SLOTS_PER_BLK),
                        we1b)

                    f_ps = pf.tile([4, FB], f32, tag="f")
                    nc.tensor.matmul(f_ps[:], ct["R1"][:],
                                     t1[:].rearrange("p n c -> p (n c)"),
                                     start=True, stop=False)
                    nc.tensor.matmul(f_ps[:], ct["R2"][:],
                                     t1u[:].rearrange("p n c -> p (n c)"),
                                     start=False, stop=True)
                    nc.scalar.copy(fstage[:, o:o + FB], f_ps[:])

                nc.sync.dma_start(out=f_d[:, cs:cs + CH], in_=fstage[:])

    nc.compile()
    return nc


def _make_runner(nc, n_cores):
    """Slim shard_map dispatch: one ExternalInput (XT), one output (f),
    consts baked into the NEFF, no zero-output upload."""
    import jax
    from jax.sharding import Mesh, PartitionSpec
    from jax.experimental.shard_map import shard_map
    from concourse import bass2jax, mybir

    bass2jax.install_neuronx_cc_hook()

    partition_name = (nc.partition_id_tensor.name
                      if nc.partition_id_tensor else None)
    in_names, out_names, out_avals = [], [], []
    for alloc in nc.m.functions[0].allocations:
        if not isinstance(alloc, mybir.MemoryLocationSet):
            continue
        name = alloc.memorylocations[0].name if alloc.memorylocations else None
        if alloc.kind == "ExternalInput":
            if name != partition_name:
                in_names.append(name)
        elif alloc.kind == "ExternalOutput":
            out_names.append(name)
            out_avals.append(jax.core.ShapedArray(
                tuple(alloc.tensor_shape), mybir.dt.np(alloc.dtype)))
    assert in_names == ["XT"] and out_names == ["f"], (in_names, out_names)
    bind_names = tuple(in_names + ([partition_name] if partition_name else []))

    def _body(xt):
        operands = [xt]
        if partition_name is not None:
            operands.append(bass2jax.partition_id_tensor())
        outs = bass2jax._bass_exec_p.bind(
            *operands,
            out_avals=tuple(out_avals),
            in_names=bind_names,
            out_names=tuple(out_names),
            lowering_input_output_aliases=(),
            sim_require_finite=True,
            sim_require_nnan=True,
            nc=nc,
        )
        return tuple(outs)

    devices = jax.devices()[:n_cores]
    mesh = Mesh(np.asarray(devices), ("core",))
    P = PartitionSpec
    fn = jax.jit(shard_map(_body, mesh=mesh, in_specs=(P("core"),),
                           out_specs=(P("core"),), check_rep=False))
    return fn


class _Result:
    def __init__(self, results):
        self.results = results
        self.exec_time_ns = None


def _get_ctx(eb, consts):
    key = (tuple(eb),
           hashlib.sha1(b"".join(np.ascontiguousarray(v).tobytes()
                                 for _, v in sorted(consts.items()))).hexdigest())
    if key not in _compiled:
        nc = _build_nc(eb, consts)
        fn = _make_runner(nc, NCORES)
        _compiled[key] = (nc, fn)
    return _compiled[key]


def kernel(**inputs):
    import jax

    x = np.asarray(inputs["node_feats"], np.float32)
    sc = np.asarray(inputs["sc"], np.float32)
    y = np.asarray(inputs["node_attrs"], np.float32)
    Wlin0 = np.asarray(inputs["Wlin0"], np.float32)
    Wlin1 = np.asarray(inputs["Wlin1"], np.float32)

    elem = np.argmax(y, axis=1)
    consts = _build_consts(inputs)

    # deal nodes: element e's nodes round-robin over cores
    count = np.bincount(elem, minlength=E)
    spe = [int(np.ceil(cnt / NCORES)) if cnt else 0 for cnt in count]
    blocks_e = [int(np.ceil(s / SLOTS_PER_BLK)) for s in spe]
    eb = []
    base_slot = []
    for e in range(E):
        base_slot.append(len(eb) * SLOTS_PER_BLK)
        eb.extend([e] * blocks_e[e])
    # pad to an XCHUNK multiple (dummy element-0 blocks, zero features)
    while len(eb) % XCHUNK:
        eb.append(0)
    NBLK = len(eb)
    NSLOT = NBLK * SLOTS_PER_BLK
    FT = NBLK * FB

    order = np.argsort(elem, kind="stable")
    core_of = np.empty(N, np.int64)
    slot_of = np.empty(N, np.int64)
    pos = 0
    for e in range(E):
        idx = order[pos:pos + count[e]]
        pos += count[e]
        j = np.arange(count[e])
        core_of[idx] = j % NCORES
        slot_of[idx] = base_slot[e] + j // NCORES

    nc, fn = _get_ctx(eb, consts)

    # XT: [9, NSLOT*C] per core, slot-major c-fast columns; dummy slots zero
    xTfull = np.concatenate(
        [x.transpose(2, 0, 1), np.zeros((I, 1, C), np.float32)], axis=1
    ).astype(np.float16)                                   # [9, N+1, C]
    node_ids = np.full((NCORES, NSLOT), N, np.int64)
    node_ids[core_of, slot_of] = np.arange(N)
    XT_all = xTfull[:, node_ids, :]                        # [9, 8, NSLOT, C]
    XT_global = np.ascontiguousarray(
        XT_all.transpose(1, 0, 2, 3).reshape(NCORES * I, FT))

    def _dispatch(xt_np):
        out = fn(xt_np)[0]
        return np.asarray(out)                             # [8*4, FT] f16

    f_global = _dispatch(XT_global)
    globals()["LAST_RESULT"] = _Result(
        [{"f": f_global[c * 4:(c + 1) * 4]} for c in range(NCORES)])

    nrep = int(os.environ.get("KERNEL_TIME_RUNS", "0"))
    if nrep:
        import time
        times = []
        for _ in range(nrep):
            t0 = time.perf_counter()
            _dispatch(XT_global)
            times.append(time.perf_counter() - t0)
        globals()["LAST_TIMES"] = times

    # inverse permutation: f_global [8*4, FT] -> f_ncd [N, C, 4]
    fg = f_global.reshape(NCORES, 4, NSLOT, C)
    f_ncd = fg[core_of, :, slot_of, :].transpose(0, 2, 1).astype(np.float32)

    inv = np.float32(1.0 / np.sqrt(C))
    y0 = (f_ncd[:, :, 0] @ Wlin0) * inv                    # [N, C]
    y1 = np.einsum("nud,uw->nwd", f_ncd[:, :, 1:], Wlin1) * inv
    out = np.concatenate([y0, y1.reshape(N, -1)], axis=-1) + sc
    return out.astype(np.float32)
